# revision 26
# baseline (speedup 1.0000x reference)
"""Trainium2 Bass kernel: 4096x4096 fp32 image, 7x7 valid cross-correlation + bias.

Strategy
--------
Column-shard the image across 8 NeuronCores: core m computes output columns
[512*m, 512*m+512) (core 7 padded; image columns padded to 4102 on host, so
every core sees an identical input shard = 512 columns + 6 halo columns).

On each core the conv runs on the tensor engine as banded-Toeplitz matmuls:
an output row band of M=122 rows uses K=128 input rows (M + kh-1) as the
moving operand and contracts against seven stationary matrices A_dj[128, 128],
A_dj[k, m] = w[k-m, dj] (zero outside the band / beyond column 121).  The
seven column taps dj become free-axis shifts of the moving operand
(rhs = x[:, dj:dj+512]) accumulated in one PSUM bank via start/stop.

Layout: the host prepacks each shard band-partition-major, xs[p, b, c] =
x[122*b + p, c], so one DMA of G=8 bands reads a contiguous 8.3 KB run per
partition (128 descriptors/transfer instead of 1024).  Same for the output.
Matmuls run dj-major across the 8 PSUM banks of a group so each stationary is
loaded once per group.  Loads issue on the Sync HWDGE ring, stores on the
GpSimd SWDGE ring, PSUM eviction (+ fused scalar bias add) on the scalar
engine.  Inputs are cast to fp16 on host (PE runs 16-bit at full rate, PSUM
accumulates fp32; fp16 keeps 11 mantissa bits -> ~3e-4 rel err).
"""

import os
import sys

import numpy as np

for _p in ("/root/.axon_site/_ro/trn_rl_repo", "/opt/trn_rl_repo"):
    if os.path.isdir(_p) and _p not in sys.path:
        sys.path.append(_p)

H = W = 4096
KH = KW = 7
OH = OW = H - KH + 1            # 4090
NCORES = 8
CW = 512                        # output columns per core
CIN = CW + KW - 1               # 518 input columns per core (incl. halo)
BAND = 128 - (KH - 1)           # 122 output rows per band
NBANDS = -(-OH // BAND)         # 34
ROWS_PAD = BAND * (NBANDS - 1) + 128    # 4154 input rows incl. zero tail
GROUP = 8                       # bands per DMA batch / PSUM-bank rotation

_prog = None


def _program():
    global _prog
    if _prog is not None:
        return _prog

    from contextlib import ExitStack

    import concourse.bass as bass
    import concourse.tile as tile
    from concourse import bacc, mybir

    nc = bacc.Bacc("TRN2", target_bir_lowering=False, debug=False)
    xs = nc.dram_tensor(
        "xs", [128, NBANDS, CIN], mybir.dt.float16, kind="ExternalInput"
    )
    ab = nc.dram_tensor("ab", [128, KW, 128], mybir.dt.float16, kind="ExternalInput")
    br = nc.dram_tensor("br", [128, 1], mybir.dt.float32, kind="ExternalInput")
    # 128 rows per band (6 zero pad rows) so the store SBUF AP keeps a
    # power-of-2 partition count -- the DGE engine spray needs it
    yd = nc.dram_tensor(
        "yd", [128, NBANDS, CW], mybir.dt.float32, kind="ExternalOutput"
    )
    xs_ap, ab_ap, br_ap, yd_ap = xs.ap(), ab.ap(), br.ap(), yd.ap()

    with tile.TileContext(nc) as tc, ExitStack() as ctx:
        consts = ctx.enter_context(tc.tile_pool(name="consts", bufs=1))
        inp = ctx.enter_context(tc.tile_pool(name="inp", bufs=3))
        pss = ctx.enter_context(tc.tile_pool(name="pss", bufs=7, space="PSUM"))
        warm = ctx.enter_context(tc.tile_pool(name="warm", bufs=1, space="PSUM"))
        outp = ctx.enter_context(tc.tile_pool(name="outp", bufs=3))

        a_t = consts.tile([128, KW, 128], mybir.dt.float16)
        nc.sync.dma_start(a_t[:, :, :], ab_ap[:, :, :])
        b_t = consts.tile([128, 1], mybir.dt.float32)
        nc.sync.dma_start(b_t[:, :], br_ap)

        # Pre-warm the PE HAM clock gate during the load phase: ~10 dummy
        # matmuls (no data deps) keep the PE busy >3.4us so the real stream
        # starts at 2.4 GHz instead of 1.2 GHz.
        junk = consts.tile([128, 128 + CW], mybir.dt.float16)
        nc.gpsimd.memset(junk[:, :], 0)
        wps = warm.tile([128, CW], mybir.dt.float32)
        for _ in range(12):
            nc.tensor.matmul(
                wps[:, :],
                junk[:, 0:128],
                junk[:, 128 : 128 + CW],
                start=True,
                stop=True,
            )

        # small first groups so the PE starts earlier; small last group so the
        # final store + serial evictions don't hang off the kernel tail
        group_sizes = [1, 2, 4, 8, 8, 8, 3]
        assert sum(group_sizes) == NBANDS
        n_groups = len(group_sizes)
        b0 = 0
        for gi, g in enumerate(group_sizes):
            xin = inp.tile([128, GROUP, CIN], mybir.dt.float16, tag="xin")
            nc.sync.dma_start(xin[:, :g, :], xs_ap[:, b0 : b0 + g, :])

            yo = outp.tile([128, GROUP, CW], mybir.dt.float32, tag="yo")
            # dj-major over subgroups of <=4 bands: one LDWEIGHTS per dj per
            # subgroup, and at most 4 PSUM banks in flight (of 8) so slot
            # turnaround never gates the PE
            for s0 in range(0, g, 4):
                sg = min(4, g - s0)
                pst = [
                    pss.tile([128, CW], mybir.dt.float32, tag="ps", name=f"ps{b0}_{i}")
                    for i in range(s0, s0 + sg)
                ]
                for dj in range(KW):
                    for k, i in enumerate(range(s0, s0 + sg)):
                        nc.tensor.matmul(
                            pst[k][:, :],
                            a_t[:, dj, :],
                            xin[:, i, dj : dj + CW],
                            start=(dj == 0),
                            stop=(dj == KW - 1),
                        )
                for k, i in enumerate(range(s0, s0 + sg)):
                    # rows 122-127 are exact zeros (A columns >= BAND are zero)
                    nc.scalar.activation(
                        yo[:, i, :],
                        pst[k][:, :],
                        mybir.ActivationFunctionType.Identity,
                        bias=b_t[:, :],
                        scale=1.0,
                    )
            st_eng = nc.sync if gi == n_groups - 1 else nc.scalar
            st_eng.dma_start(yd_ap[:, b0 : b0 + g, :], yo[:, :g, :])
            b0 += g

    nc.compile()
    _prog = nc
    return nc


def _shards(x, weight, bias):
    x = np.asarray(x, dtype=np.float32)
    weight = np.asarray(weight, dtype=np.float32)
    bias = np.asarray(bias, dtype=np.float32)

    xp = np.zeros((ROWS_PAD, NCORES * CW + (KW - 1)), dtype=np.float16)
    xp[:H, :W] = x.astype(np.float16)

    wh = weight.astype(np.float16)
    abm = np.zeros((128, KW, 128), dtype=np.float16)
    idx = np.arange(BAND)
    for dj in range(KW):
        for di in range(KH):
            abm[idx + di, dj, idx] = wh[di, dj]

    brep = np.full((128, 1), np.float32(bias[0]), dtype=np.float32)

    s0, s1 = xp.strides
    ins = []
    for m in range(NCORES):
        core = xp[:, m * CW : m * CW + CIN]
        # xs[p, b, c] = core[BAND*b + p, c] -- overlapping-band strided view
        xb = np.lib.stride_tricks.as_strided(
            core, shape=(128, NBANDS, CIN), strides=(s0, BAND * s0, s1)
        )
        ins.append({"xs": np.ascontiguousarray(xb), "ab": abm, "br": brep})
    return ins


def _gather(results):
    y = np.empty((OH, OW), dtype=np.float32)
    for m in range(NCORES):
        c0 = m * CW
        c1 = min(c0 + CW, OW)
        # yd[r, b, c] = out[BAND*b + r, c] for r < BAND; rows >= BAND are pad
        full = results[m]["yd"].transpose(1, 0, 2)[:, :BAND, :].reshape(
            BAND * NBANDS, CW
        )
        y[:, c0:c1] = full[:OH, : c1 - c0]
    return y


def kernel(x, weight, bias):
    from concourse.bass_utils import run_bass_kernel_spmd

    nc = _program()
    in_maps = _shards(x, weight, bias)
    res = run_bass_kernel_spmd(nc, in_maps, core_ids=list(range(NCORES)))
    return _gather(res.results)
